# revision 41
# baseline (speedup 1.0000x reference)
"""DepthConsistencyLoss Trainium2 kernel (8 NeuronCores, batch-parallel).

loss = mean_{n,l} sum_{r=0..188} w_{r%9}[l] * (cam_unfold[r,l] - cam_center[r%21,l])^2

Restructure (loss*N*H*W = sum_n T1 - 2*T2 + 3*T3'):
  Key identity: S_{-dp} w_p = w_{8-p} (spatial weights symmetric), so with
  masked weights w~_q = w_q * [l + dq inside image]:
    T1 = sum_l E * W~tot            E = sum_c cam_c^2, W~tot = sum_q w~_q
    T2 = sum_g sum_l R~_{2-g} * Pi_g   R~_h = row sums of w~,
         Pi_g = sum_{c0} P_c0 * S_{(dy,0)} cam_{c'}  (13 distinct products)
    T3' = sum_{c'} G_{c'} * Om_{c'}    Om from vertical band-matmuls of
         X = S_(0,1)wsum_0 + wsum_1 + S_(0,-1)wsum_2
  Masking is free: the depth windows are host-padded with BIG=1e4, so
  invalid-shift weights come out exp(-50*BIG^2) = 0.

Layout: 2 tiles of 114 partitions = image rows; tile0 rows k=0..113,
tile1 REVERSED rows 223-k (so per-tile outputs m=0..111 start at partition 0
on both tiles). y-shifts run on the otherwise idle PE with [114,112]
shift/band matrices; x-shifts are free-dim AP offsets. The host pre-packs
per-tile partition-major arrays (bf16) so each DMA descriptor moves >=512B
contiguous (avoids the small-descriptor bandwidth penalty).
Each core does one batch element; host sums the 8 x [2,112,5] partials.
"""

import os
import sys

import numpy as np

for _p in ("/opt/trn_rl_repo", os.path.expanduser("~/.axon_site/_ro/trn_rl_repo")):
    if os.path.isdir(_p) and _p not in sys.path:
        sys.path.insert(0, _p)

import concourse.bass as bass
import concourse.bacc as bacc
import concourse.tile as tile
from concourse import mybir
from concourse.bass_utils import run_bass_kernel_spmd

F32 = mybir.dt.float32
BF16 = mybir.dt.bfloat16
Alu = mybir.AluOpType
Act = mybir.ActivationFunctionType

N, C, H, W = 8, 21, 224, 224
KP = 114          # k-space partitions per tile (rows + 2 halo for PE shifts)
MP = 112          # m-space output rows per tile
XF = 228          # padded depth row: [2 pad][224][2 pad]
X0, X1 = 2, 226
NACC = 3
BIG = 1.0e4

# 13 distinct products (dy, c0, c'), dy-major; runs of consecutive c0/c'.
PRODS = [(-2, 0, 10), (-2, 1, 11),
         (-1, 4, 9), (-1, 5, 10), (-1, 6, 11),
         (0, 2, 9), (0, 3, 10), (0, 4, 11),
         (1, 0, 9), (1, 1, 10), (1, 2, 11),
         (2, 5, 9), (2, 6, 10)]
GROUPS = {0: [8, 9, 10, 6, 7, 11, 12],
          1: [8, 9, 5, 6, 7, 3, 4],
          2: [0, 1, 5, 6, 2, 3, 4]}
# R-combo slot per product (sum of R~_{2-g} over the groups using it):
# slots in rsc: 0=R~_0, 1=R~_0+R~_1, 2=W~tot, 3=R~_1+R~_2, 4=R~_2
UCOMBO = [0, 0, 0, 1, 1, 1, 2, 3, 3, 3, 4, 4, 4]
# U-mult batches: (prod_idx0, n, combo_slot)  [c0 runs with constant combo]
UBATCH = [(0, 2, 0), (2, 1, 0), (3, 2, 1), (5, 1, 1), (6, 1, 2),
          (7, 1, 3), (8, 2, 3), (10, 1, 4), (11, 2, 4)]
# Theta accumulation target per product: c' 9 -> 0, 10 -> 1, 11 -> 2
THETA = [cp - 9 for (dy, c0, cp) in PRODS]

MAT_B = {dy: 2 + dy for dy in (-2, -1, 0, 1, 2)}   # B_j at slot 2+j
MAT_DM, MAT_DT, MAT_DP = 5, 6, 7                   # B0+B-1, tri, B0+B+1
NMAT = 8


def _build_wm():
    """[KP, NMAT, MP] bf16 shift/band matrices W[k, m] (partition-major)."""
    wm = np.zeros((NMAT, KP, MP), np.float32)
    for j in (-2, -1, 0, 1, 2):
        for m in range(MP):
            k = m + j
            if 0 <= k < KP:
                wm[MAT_B[j], k, m] = 1.0
    wm[MAT_DM] = wm[MAT_B[0]] + wm[MAT_B[-1]]
    wm[MAT_DT] = wm[MAT_B[-1]] + wm[MAT_B[0]] + wm[MAT_B[1]]
    wm[MAT_DP] = wm[MAT_B[0]] + wm[MAT_B[1]]
    return np.ascontiguousarray(wm.transpose(1, 0, 2))


class _T:
    """Per-tile SBUF buffers."""

    def __init__(self, pool, t):
        self.t = t
        self.camb = pool.tile([KP, C, W], BF16, name=f"camb{t}", tag=f"camb{t}")
        self.dsh = pool.tile([KP, 9, XF], BF16, name=f"dsh{t}", tag=f"dsh{t}")
        self.gsq = pool.tile([MP, C, W], BF16, name=f"gsq{t}", tag=f"gsq{t}")
        self.ddif = pool.tile([KP, 9, XF], BF16, name=f"ddif{t}", tag=f"ddif{t}")
        self.dsq = pool.tile([KP, 9, XF], BF16, name=f"dsq{t}", tag=f"dsq{t}")
        self.wb = pool.tile([KP, 9, XF], BF16, name=f"wb{t}", tag=f"wb{t}")
        self.wsum = pool.tile([KP, 3, XF], BF16, name=f"wsum{t}", tag=f"wsum{t}")
        self.xb = pool.tile([KP, XF], BF16, name=f"xb{t}", tag=f"xb{t}")
        self.wtot = pool.tile([KP, XF], BF16, name=f"wtot{t}", tag=f"wtot{t}")
        self.rb = pool.tile([KP, 3, XF], BF16, name=f"rb{t}", tag=f"rb{t}")
        self.rsc = pool.tile([KP, 2, XF], BF16, name=f"rsc{t}", tag=f"rsc{t}")
        self.pb = pool.tile([KP, 7, W], BF16, name=f"pb{t}", tag=f"pb{t}")
        self.ub = pool.tile([KP, 13, W], BF16, name=f"ub{t}", tag=f"ub{t}")
        self.scr = pool.tile([MP, 3, W], BF16, name=f"scr{t}", tag=f"scr{t}")
        self.acc = pool.tile([MP, NACC], F32, name=f"acc{t}", tag=f"acc{t}")


class _PS:
    """Shared PSUM tiles, one bank each (stride-256 keeps matmul outs
    in-bank); E double-buffered per tile, the rest reused (WAR syncs)."""

    def __init__(self, ppool):
        self.e0 = ppool.tile([MP, 256], F32, name="e0", tag="e0")
        self.e1 = ppool.tile([MP, 256], F32, name="e1", tag="e1")
        self.e = {0: self.e0[:, 0:W], 1: self.e1[:, 0:W]}
        # per-tile 3-bank tile: slots 0-2 = Om_9..11, slots 3-5 = Theta_9..11
        # (each 224-wide matmul out region stays inside one 2KB bank)
        self.po6 = {0: ppool.tile([MP, 6, 256], F32, name="po6_0", tag="po6_0"),
                    1: ppool.tile([MP, 6, 256], F32, name="po6_1", tag="po6_1")}

    def theta(self, t, i):
        return self.po6[t][:, 3 + i, 0:W]

    def om(self, t, g):
        return self.po6[t][:, g, 0:W]


def _emit_cam_chunk(nc, b, ct, c0, c1):
    """camb[:, c0:c1, :] from the per-tile partition-major DRAM image."""
    nc.sync.dma_start(
        out=b.camb[:, c0:c1, :],
        in_=bass.AP(ct, c0 * W, [[C * W, KP], [W, c1 - c0], [1, W]]))


def _emit_dsh_load(nc, b, dsh, q0, q1):
    nc.sync.dma_start(
        out=b.dsh[:, q0:q1, :],
        in_=bass.AP(dsh, q0 * XF, [[9 * XF, KP], [XF, q1 - q0], [1, XF]]))


def _emit_ddif(nc, b, t, q0, q1, eng=None):
    """ddif = dsh9 - center (bf16), q-range piece. The q=4 lane is
    computed but unused (w~_4 is the constant 1)."""
    e = nc.gpsimd if eng == "pool" else nc.vector
    dst = b.dsh.ap[0][0]
    in1 = bass.AP(b.dsh.tensor, b.dsh.offset + 4 * XF + 1,
                  [[dst, KP], [0, q1 - q0], [1, 226]])
    e.tensor_tensor(out=b.ddif[:, q0:q1, 1:227], in0=b.dsh[:, q0:q1, 1:227],
                    in1=in1, op=Alu.subtract)


def _emit_dsq(nc, b, t, eng=None):
    """dsq = ddif^2 (DVE 2x or ACT Square)."""
    if eng == "act":
        nc.scalar.activation(out=b.dsq[:, :, 1:227], in_=b.ddif[:, :, 1:227],
                             func=Act.Square)
    else:
        nc.vector.tensor_tensor(out=b.dsq[:, :, 1:227],
                                in0=b.ddif[:, :, 1:227],
                                in1=b.ddif[:, :, 1:227], op=Alu.mult)


def _emit_exp(nc, b, t, bias2):
    """ACT: w~ = exp(-50*dsq + ln wspat)  (4 instrs by wspat class)."""
    s = nc.scalar
    s.activation(out=b.wb[:, 1:8:2, 1:227], in_=b.dsq[:, 1:8:2, 1:227],
                 func=Act.Exp, scale=-50.0, bias=bias2[:, 0:1])
    s.activation(out=b.wb[:, 0:3:2, 1:227], in_=b.dsq[:, 0:3:2, 1:227],
                 func=Act.Exp, scale=-50.0, bias=bias2[:, 1:2])
    s.activation(out=b.wb[:, 6:9:2, 1:227], in_=b.dsq[:, 6:9:2, 1:227],
                 func=Act.Exp, scale=-50.0, bias=bias2[:, 1:2])


def _emit_wsum(nc, b, t):
    """DVE: wsum_m = w_m + w_{m+3} + w_{m+6}."""
    v = nc.vector
    v.tensor_tensor(out=b.wsum[:, :, 1:227], in0=b.wb[:, 0:3, 1:227],
                    in1=b.wb[:, 3:6, 1:227], op=Alu.add)
    v.tensor_tensor(out=b.wsum[:, :, 1:227], in0=b.wsum[:, :, 1:227],
                    in1=b.wb[:, 6:9, 1:227], op=Alu.add)


def _emit_X(nc, b, t):
    """DVE: X from wsum with free x-shifts."""
    v = nc.vector
    v.tensor_tensor(out=b.xb[:, X0:X1], in0=b.wsum[:, 0, X0 + 1:X1 + 1],
                    in1=b.wsum[:, 1, X0:X1], op=Alu.add)
    v.tensor_tensor(out=b.xb[:, X0:X1], in0=b.xb[:, X0:X1],
                    in1=b.wsum[:, 2, X0 - 1:X1 - 1], op=Alu.add)


def _emit_pool_w(nc, b, t):
    """Pool: R (from w~), Wtot (from wsum), RS combos -- all on KP rows."""
    g = nc.gpsimd
    wst = b.wb.ap[0][0]
    w_s3 = lambda q0, xoff: bass.AP(b.wb.tensor, b.wb.offset + q0 * XF + xoff,
                                    [[wst, KP], [3 * XF, 3], [1, 224]])
    g.tensor_tensor(out=b.rb[:, :, X0:X1], in0=w_s3(0, X0), in1=w_s3(1, X0),
                    op=Alu.add)
    g.tensor_tensor(out=b.rb[:, :, X0:X1], in0=b.rb[:, :, X0:X1],
                    in1=w_s3(2, X0), op=Alu.add)
    g.tensor_tensor(out=b.wtot[:, X0:X1], in0=b.wsum[:, 0, X0:X1],
                    in1=b.wsum[:, 1, X0:X1], op=Alu.add)
    g.tensor_tensor(out=b.wtot[:, X0:X1], in0=b.wtot[:, X0:X1],
                    in1=b.wsum[:, 2, X0:X1], op=Alu.add)
    # rsc[0] = R0+R1, rsc[1] = R1+R2
    g.tensor_tensor(out=b.rsc[:, :, X0:X1], in0=b.rb[:, 0:2, X0:X1],
                    in1=b.rb[:, 1:3, X0:X1], op=Alu.add)


def _mm(nc, wmb, out, mat_slot, rhs, start, stop, kp=KP):
    nc.tensor.matmul(out=out, lhsT=wmb[0:kp, mat_slot, :], rhs=rhs,
                     start=start, stop=stop)


def _emit_sq_act(nc, b, t, rng):
    nc.scalar.activation(out=b.gsq[:, rng[0]:rng[1], :],
                         in_=b.camb[0:MP, rng[0]:rng[1], :], func=Act.Square)


def _emit_sq_dve(nc, b, t, rng):
    nc.vector.tensor_tensor(out=b.gsq[:, rng[0]:rng[1], :],
                            in0=b.camb[0:MP, rng[0]:rng[1], :],
                            in1=b.camb[0:MP, rng[0]:rng[1], :], op=Alu.mult)


def _emit_P1(nc, b, t, eng=None):
    """P partial: cam[7:14] + cam[14:21] (chunks B+C), on all KP rows."""
    e = nc.gpsimd if eng == "pool" else nc.vector
    e.tensor_tensor(out=b.pb[:, :, :], in0=b.camb[:, 7:14, :],
                    in1=b.camb[:, 14:21, :], op=Alu.add)


def _emit_P2(nc, b, t):
    """P += cam[0:7] (chunk A)."""
    nc.vector.tensor_tensor(out=b.pb[:, :, :], in0=b.pb[:, :, :],
                            in1=b.camb[:, 0:7, :], op=Alu.add)


def _emit_U(nc, b, t):
    """DVE: U_pr = Rcombo_pr * P_c0 (9 batched bf16 2x mults)."""
    v = nc.vector
    combos = {0: (b.rb, 0), 1: (b.rsc, 0), 2: (b.wtot, None),
              3: (b.rsc, 1), 4: (b.rb, 2)}
    for i0, n, cs in UBATCH:
        dy, c0, cp = PRODS[i0]
        buf, img = combos[cs]
        st = buf.ap[0][0]
        off = buf.offset + (0 if img is None else img * XF) + X0
        in0 = bass.AP(buf.tensor, off, [[st, KP], [0, n], [1, 224]])
        v.tensor_tensor(out=b.ub[:, i0:i0 + n, :],
                        in0=in0, in1=b.pb[:, c0:c0 + n, :], op=Alu.mult)


def _emit_Theta(nc, b, ps, t, wmb):
    """PE: Theta_c' += S_{-dy} U_pr (13 shifted accumulation passes).
    Each bank's accumulation group runs consecutively (interleaving groups
    within a PSUM bank corrupts the accumulation)."""
    sgn = 1 if t == 0 else -1
    for cpi in (0, 1, 2):
        idxs = [i for i, c in enumerate(THETA) if c == cpi]
        for j, i in enumerate(idxs):
            dy = PRODS[i][0]
            _mm(nc, wmb, ps.theta(t, cpi), MAT_B[-sgn * dy], b.ub[:, i, :],
                start=(j == 0), stop=(j == len(idxs) - 1))


def _emit_E(nc, b, ps, t, wmb, crng, first=False, last=False):
    c0, c1 = crng
    for c in range(c0, c1):
        _mm(nc, wmb, ps.e[t], MAT_B[0], b.gsq[:, c, :],
            start=(first and c == c0), stop=(last and c == c1 - 1), kp=MP)


def _emit_om(nc, b, ps, t, wmb):
    om_mats = (MAT_DM, MAT_DT, MAT_DP) if t == 0 else (MAT_DP, MAT_DT, MAT_DM)
    for g in range(3):
        _mm(nc, wmb, ps.om(t, g), om_mats[g], b.xb[:, X0:X1], start=True,
            stop=True)


def _emit_amr_om(nc, b, ps, t):
    """T3 in one amr: 3*sum gsq_{9:12} * Om."""
    nc.vector.affine_mul_reduce(out=b.scr[:, 0:3, :], accum_out=b.acc[:, 2:3],
                                in0=b.gsq[:, 9:12, :],
                                in1=ps.po6[t][:, 0:3, 0:W],
                                scale=3.0, bias=0.0)


def _emit_amr_T1(nc, b, ps, t):
    nc.vector.affine_mul_reduce(out=b.scr[:, 0, :], accum_out=b.acc[:, 0:1],
                                in0=b.wtot[0:MP, X0:X1], in1=ps.e[t],
                                scale=1.0, bias=0.0)


def _emit_amr_T2(nc, b, ps, t, out):
    """T2 in one amr: -2*sum cam_{9:12} * Theta; then ship acc."""
    nc.vector.affine_mul_reduce(out=b.scr[:, 0:3, :], accum_out=b.acc[:, 1:2],
                                in0=b.camb[0:MP, 9:12, :],
                                in1=ps.po6[t][:, 3:6, 0:W],
                                scale=-2.0, bias=0.0)
    nc.sync.dma_start(out=out[t], in_=b.acc[:, :])


def build_nc():
    nc = bacc.Bacc("TRN2", target_bir_lowering=False)
    ct = {0: nc.dram_tensor("ct0", (KP, C, W), BF16, kind="ExternalInput"),
          1: nc.dram_tensor("ct1", (KP, C, W), BF16, kind="ExternalInput")}
    dsh = {0: nc.dram_tensor("dsh0", (KP, 9, XF), BF16, kind="ExternalInput"),
           1: nc.dram_tensor("dsh1", (KP, 9, XF), BF16, kind="ExternalInput")}
    wm = nc.dram_tensor("wm", (KP, NMAT, MP), BF16, kind="ExternalInput")
    out = nc.dram_tensor("out", (2, MP, NACC), F32, kind="ExternalOutput")
    with tile.TileContext(nc) as tc:
        with tc.tile_pool(name="main", bufs=1) as pool, \
             tc.tile_pool(name="psum", bufs=1, space="PSUM") as ppool:
            wmb = pool.tile([KP, NMAT, MP], BF16, name="wmb", tag="wmb")
            bias2 = pool.tile([KP, 2], F32, name="bias2s", tag="bias2s")
            warm = pool.tile([KP, 2], BF16, name="warm", tag="warm")
            bs = {t: _T(pool, t) for t in (0, 1)}
            ps = _PS(ppool)
            b0, b1 = bs[0], bs[1]

            # Pool: bias consts, constant w~_4 = 1; ACT: warm the Exp table
            nc.gpsimd.memset(bias2[:, 0:1], -0.02 * 1.0)
            nc.gpsimd.memset(bias2[:, 1:2], -0.02 * 2.0)
            nc.scalar.activation(out=warm[:, :], in_=bias2[:, :],
                                 func=Act.Exp)
            nc.gpsimd.memset(b0.wb[:, 4, 1:227], 1.0)
            nc.gpsimd.memset(b1.wb[:, 4, 1:227], 1.0)

            # DMA queue: dsh thirds (center first), cam chunks B, C, A, wm
            for q0, q1 in ((3, 5), (5, 9), (0, 3)):
                _emit_dsh_load(nc, b0, dsh[0], q0, q1)
            for q0, q1 in ((3, 5), (5, 9), (0, 3)):
                _emit_dsh_load(nc, b1, dsh[1], q0, q1)
            nc.sync.dma_start(
                out=wmb[:, :, :],
                in_=bass.AP(wm, 0, [[NMAT * MP, KP], [MP, NMAT], [1, MP]]))
            for (c0, c1), tt in (((7, 14), 0), ((14, 21), 0), ((7, 14), 1),
                                 ((14, 21), 1), ((0, 7), 0), ((0, 7), 1)):
                _emit_cam_chunk(nc, bs[tt], ct[tt], c0, c1)

            # DVE: depth diffs + squares (center piece first)
            _emit_ddif(nc, b0, 0, 3, 5)
            _emit_ddif(nc, b0, 0, 5, 9)
            _emit_ddif(nc, b0, 0, 0, 3)
            _emit_dsq(nc, b0, 0)
            _emit_ddif(nc, b1, 1, 3, 5)
            _emit_ddif(nc, b1, 1, 5, 9)
            _emit_ddif(nc, b1, 1, 0, 3)
            _emit_dsq(nc, b1, 1)
            # ACT: exp asap
            _emit_exp(nc, b0, 0, bias2)
            _emit_exp(nc, b1, 1, bias2)
            # Pool: wsum + R; DVE: P, X
            _emit_wsum(nc, b0, 0)
            _emit_P1(nc, b0, 0)
            _emit_P1(nc, b1, 1)
            _emit_X(nc, b0, 0)
            _emit_pool_w(nc, b0, 0)
            _emit_P2(nc, b0, 0)
            _emit_wsum(nc, b1, 1)
            _emit_X(nc, b1, 1)
            _emit_P2(nc, b1, 1)
            _emit_pool_w(nc, b1, 1)
            # ACT squares for chunk B (feed amr_om and E)
            _emit_sq_act(nc, b0, 0, (7, 14))
            _emit_sq_act(nc, b1, 1, (7, 14))
            # DVE: U products; PE: om, Theta
            _emit_om(nc, b0, ps, 0, wmb)
            _emit_om(nc, b1, ps, 1, wmb)
            _emit_U(nc, b0, 0)
            _emit_amr_om(nc, b0, ps, 0)
            _emit_Theta(nc, b0, ps, 0, wmb)
            _emit_U(nc, b1, 1)
            _emit_amr_om(nc, b1, ps, 1)
            _emit_Theta(nc, b1, ps, 1, wmb)
            # ACT squares chunks C, A (tile1 first: its amr chain closes first)
            _emit_sq_act(nc, b0, 0, (14, 21))
            _emit_sq_act(nc, b1, 1, (14, 21))
            _emit_sq_act(nc, b1, 1, (0, 7))
            _emit_sq_act(nc, b0, 0, (0, 7))
            # PE: E ranges interleaved across the two banks as squares land;
            # DVE: final amrs (tile1 chain closes first, tile0 last)
            _emit_E(nc, b0, ps, 0, wmb, (7, 14), first=True)
            _emit_E(nc, b1, ps, 1, wmb, (7, 14), first=True)
            _emit_E(nc, b0, ps, 0, wmb, (14, 21))
            _emit_E(nc, b1, ps, 1, wmb, (14, 21))
            _emit_E(nc, b1, ps, 1, wmb, (0, 7), last=True)
            _emit_amr_T1(nc, b1, ps, 1)
            _emit_amr_T2(nc, b1, ps, 1, out)
            _emit_E(nc, b0, ps, 0, wmb, (0, 7), last=True)
            _emit_amr_T1(nc, b0, ps, 0)
            _emit_amr_T2(nc, b0, ps, 0, out)
    nc.finalize()
    return nc


_CACHE = {}


def _get_nc():
    if "nc" not in _CACHE:
        _CACHE["nc"] = build_nc()
    return _CACHE["nc"]


def _run(in_maps, **kw):
    return run_bass_kernel_spmd(_get_nc(), in_maps, core_ids=list(range(N)), **kw)


def _make_in_maps(cam_map, depth_map):
    import ml_dtypes

    BF = ml_dtypes.bfloat16
    camb = np.ascontiguousarray(cam_map, dtype=np.float32).astype(BF)
    dep = np.ascontiguousarray(depth_map, dtype=np.float32)[:, 0]  # (N,H,W)

    # per-tile partition-major cam: tile0 rows 0..113, tile1 rows 223..110
    ct0 = np.ascontiguousarray(camb[:, :, 0:KP, :].transpose(0, 2, 1, 3))
    ct1 = np.ascontiguousarray(
        camb[:, :, H - 1:H - 1 - KP:-1, :].transpose(0, 2, 1, 3))

    # depth shift-windows with BIG padding: slot q holds D(r + dy_q, x + dx_q)
    # so ddif is a single subtract on-chip.
    def build_dsh(rows):                       # rows: length-KP image rows
        out = np.full((N, KP, 9, XF), BIG, np.float32)
        for ki, r in enumerate(rows):
            for q in range(9):
                dy, dx = q // 3 - 1, q % 3 - 1
                rr = r + dy
                if not (0 <= rr < H):
                    continue
                # buffer col x holds D(rr, (x - X0) + dx): cols [X0-dx, X0-dx+W)
                out[:, ki, q, X0 - dx:X0 - dx + W] = dep[:, rr, :]
        return out.astype(BF)

    dsh0 = build_dsh(list(range(0, KP)))
    dsh1 = build_dsh(list(range(H - 1, H - 1 - KP, -1)))
    wmh = _build_wm().astype(BF)
    return [{"ct0": ct0[i], "ct1": ct1[i], "dsh0": dsh0[i], "dsh1": dsh1[i],
             "wm": wmh} for i in range(N)]


def kernel(cam_map, depth_map):
    r = _run(_make_in_maps(cam_map, depth_map))
    tot = sum(float(m["out"].astype(np.float64).sum()) for m in r.results)
    return np.array(tot / (N * H * W), dtype=np.float32)
